# revision 60
# baseline (speedup 1.0000x reference)
"""Trainium2 Bass kernel for nn_DenseFusionLoss (DenseFusion pose-estimation loss).

Strategy: data-parallel over the batch axis. Each of the 8 NeuronCores gets 4
batches (poses/confidences/class_ids shard) plus the full replicated
[21,2048,3] vertex table. Each core computes partial sums
[sum_selected_add_loss, sum_softplus, sum_pose_reg]; the host combines the 8
partial vectors into the final scalar loss.

Device-side computation per core (all heavy math on-device):
  - quat -> rotation matrices via unnormalized-product form (scaled by 1/|q|^2)
  - vertex gather fused into the point-transform matmul: lhsT[(c,i), d] =
    onehot[c] * R[d,i] (K=63), rhs = vertex table laid out [63, 2048]
  - pairwise squared distances d2[v,w] = pn[v] + gn[w] - 2 p.g via a K=5
    matmul with lhsT rows [-2px,-2py,-2pz, pn, 1], rhs rows [gx,gy,gz, 1, gn]
  - ADD-S: DVE reduce_min over PSUM d2 tiles, clamp, sqrt, mean
  - ADD: true-difference form on GPSIMD + PE partition-sum + ACT sqrt-accum
  - conf loss: softplus(-x) = Ln(1 + Exp(-x)) with ACT accumulate
  - pose reg: relu(|t|-2)^2 via ACT
"""

from contextlib import ExitStack

import numpy as np

import concourse.bass as bass
import concourse.bacc as bacc
import concourse.tile as tile
from concourse import mybir
from concourse.bass_utils import run_bass_kernel_spmd

B, C, V, NCONF = 32, 21, 2048, 1024
NCORES = 8
BPC = B // NCORES  # batches per core
F32 = mybir.dt.float32
F16 = mybir.dt.float16
I32 = mybir.dt.int32
AF = mybir.ActivationFunctionType
OP = mybir.AluOpType
AX = mybir.AxisListType

ADD_WEIGHT = 1.0
CONF_WEIGHT = 0.1
POSE_REG_WEIGHT = 0.1

_CACHE = {}


def _emit(nc, tc, h, ctx):
    pool = {}
    pool["setup"] = ctx.enter_context(tc.tile_pool(name="setup", bufs=1))
    pool["acc"] = ctx.enter_context(tc.tile_pool(name="acc", bufs=1))
    pool["ab"] = ctx.enter_context(tc.tile_pool(name="ab", bufs=2))
    pool["work"] = ctx.enter_context(tc.tile_pool(name="work", bufs=2))
    pool["psB"] = ctx.enter_context(tc.tile_pool(name="psB", bufs=3, space="PSUM"))
    pool["psS"] = ctx.enter_context(tc.tile_pool(name="psS", bufs=2, space="PSUM"))
    pool["dram"] = ctx.enter_context(tc.tile_pool(name="dram", bufs=2, space="DRAM"))

    setup = pool["setup"]
    acc = pool["acc"]
    work = pool["work"]
    psS = pool["psS"]
    psB = pool["psB"]

    # ---------------- constant / input loads ----------------
    iota21 = setup.tile([21, 1], F32, tag="iota21")
    nc.sync.dma_start(out=iota21, in_=h["iota21"].ap())
    ones = setup.tile([128, 1], F32, tag="ones")
    nc.sync.dma_start(out=ones, in_=h["ones"].ap())



    poses = setup.tile([8, 7], F32, tag="poses")
    nc.sync.dma_start(out=poses, in_=h["poses"].ap())

    # t5[d, j] = poses[j, d] for d in 0..4 (rows 3-4 junk, masked to 0 below)
    t5 = setup.tile([5, 8], F32, tag="t5")
    for j in range(8):
        nc.sync.dma_start(
            out=t5[:, j : j + 1],
            in_=bass.AP(tensor=h["poses"].ap().tensor, offset=j * 7, ap=[[1, 5]]),
        )
    mask_a = setup.tile([5, 1], F32, tag="mask_a")
    nc.sync.dma_start(out=mask_a, in_=h["mask_a"].ap())
    mask_g = setup.tile([5, 1], F32, tag="mask_g")
    nc.sync.dma_start(out=mask_g, in_=h["mask_g"].ap())
    # fill matmul lhsT constants: e3x5_rK sums 3 rows into psum row K
    e3x5_r3 = setup.tile([3, 5], F16, tag="e3x5_r3")
    nc.sync.dma_start(out=e3x5_r3, in_=h["e3x5_r3"].ap())
    e3x5_r4 = setup.tile([3, 5], F16, tag="e3x5_r4")
    nc.sync.dma_start(out=e3x5_r4, in_=h["e3x5_r4"].ap())
    ones3h = setup.tile([3, 1], F16, tag="ones3h")
    nc.sync.dma_start(out=ones3h, in_=h["ones3h"].ap())
    # per-row copy-out scale (zero on the constant-ones row) and bias
    # columns: rows 0-2 = (-2|1)*t, ones row = 1, norm row = 0
    scale_a = setup.tile([5, 1], F32, tag="scale_a")
    nc.sync.dma_start(out=scale_a, in_=h["scale_a"].ap())
    scale_g = setup.tile([5, 1], F32, tag="scale_g")
    nc.sync.dma_start(out=scale_g, in_=h["scale_g"].ap())
    addv_a = setup.tile([5, 1], F32, tag="addv_a")
    nc.sync.dma_start(out=addv_a, in_=h["addv_a"].ap())
    addv_g = setup.tile([5, 1], F32, tag="addv_g")
    nc.sync.dma_start(out=addv_g, in_=h["addv_g"].ap())
    bias_a = setup.tile([5, 8], F32, tag="bias_a")
    nc.vector.tensor_scalar(
        out=bias_a, in0=t5, scalar1=mask_a, scalar2=addv_a,
        op0=OP.mult, op1=OP.add,
    )
    bias_g = setup.tile([5, 8], F32, tag="bias_g")
    nc.vector.tensor_scalar(
        out=bias_g, in0=t5, scalar1=mask_g, scalar2=addv_g,
        op0=OP.mult, op1=OP.add,
    )

    conf = setup.tile([BPC, NCONF], F32, tag="conf")
    nc.sync.dma_start(out=conf, in_=h["conf"].ap())

    # ---------------- quaternion -> rotation matrices ----------------
    q = poses[:, 3:7]
    qsq = setup.tile([8, 4], F32, tag="qsq")
    nc.vector.tensor_mul(qsq, q, q)
    nrm2 = setup.tile([8, 1], F32, tag="nrm2")
    nc.vector.tensor_reduce(out=nrm2, in_=qsq, axis=AX.X, op=OP.add)
    inv2 = setup.tile([8, 1], F32, tag="inv2")
    nc.vector.reciprocal(inv2, nrm2)
    s2 = setup.tile([8, 1], F32, tag="s2")
    nc.vector.tensor_scalar_mul(s2, inv2, 2.0)
    ns2 = setup.tile([8, 1], F32, tag="ns2")
    nc.vector.tensor_scalar_mul(ns2, inv2, -2.0)

    # cross products: xy xz yz wx wy wz
    pr = setup.tile([8, 6], F32, tag="pr")
    nc.vector.tensor_mul(pr[:, 0:1], q[:, 1:2], q[:, 2:3])  # xy
    nc.vector.tensor_mul(pr[:, 1:2], q[:, 1:2], q[:, 3:4])  # xz
    nc.vector.tensor_mul(pr[:, 2:3], q[:, 2:3], q[:, 3:4])  # yz
    nc.vector.tensor_mul(pr[:, 3:4], q[:, 0:1], q[:, 1:2])  # wx
    nc.vector.tensor_mul(pr[:, 4:5], q[:, 0:1], q[:, 2:3])  # wy
    nc.vector.tensor_mul(pr[:, 5:6], q[:, 0:1], q[:, 3:4])  # wz

    xx, yy, zz = qsq[:, 1:2], qsq[:, 2:3], qsq[:, 3:4]
    xy, xz, yz = pr[:, 0:1], pr[:, 1:2], pr[:, 2:3]
    wx, wy, wz = pr[:, 3:4], pr[:, 4:5], pr[:, 5:6]

    sm = setup.tile([8, 9], F32, tag="sm")
    # entry order is column-major: e = i*3 + d holds R[d][i], so that
    # (b, d)-indexed DMA reads of r_dram have unit stride in d.
    # entry: (a op b); diag entries get ns2*sum + 1, off-diag s2*sum
    entries = [
        (yy, zz, OP.add, True),   # e=0: R00 = 1 - 2(yy+zz)/n2
        (xy, wz, OP.add, False),  # e=1: R10
        (xz, wy, OP.subtract, False),  # e=2: R20
        (xy, wz, OP.subtract, False),  # e=3: R01
        (xx, zz, OP.add, True),   # e=4: R11
        (yz, wx, OP.add, False),  # e=5: R21
        (xz, wy, OP.add, False),  # e=6: R02
        (yz, wx, OP.subtract, False),  # e=7: R12
        (xx, yy, OP.add, True),   # e=8: R22
    ]
    r_all = setup.tile([8, 9], F32, tag="r_all")
    for e, (a, b_, op, diag) in enumerate(entries):
        nc.vector.tensor_tensor(out=sm[:, e : e + 1], in0=a, in1=b_, op=op)
        nc.vector.tensor_scalar(
            out=r_all[:, e : e + 1],
            in0=sm[:, e : e + 1],
            scalar1=ns2 if diag else s2,
            scalar2=1.0 if diag else 0.0,
            op0=OP.mult,
            op1=OP.add,
        )

    # ---------------- one-hot class rows ----------------
    cls21 = setup.tile([21, 4], I32, tag="cls21")
    nc.gpsimd.dma_start(
        out=cls21,
        in_=bass.AP(tensor=h["cls"].ap().tensor, offset=0, ap=[[0, 21], [1, 4]]),
    )
    cls21f = setup.tile([21, 4], F32, tag="cls21f")
    nc.vector.tensor_copy(out=cls21f, in_=cls21)
    oh21 = setup.tile([21, 4], F32, tag="oh21")
    nc.vector.tensor_scalar(
        out=oh21, in0=cls21f, scalar1=iota21, scalar2=None, op0=OP.is_equal
    )

    # ---------------- replicated transform lhsT ----------------
    # l120[c, col], col = side*60 + b*15 + i*5 + d (d-slots of width 5, the
    # last two zero) holds onehot_b(c) * R_side,b[d, i] (pred side * -2)
    r_dram = pool["dram"].tile([8, 9], F32, tag="r_dram")
    nc.sync.dma_start(out=r_dram, in_=r_all)
    l120 = setup.tile([C, 120], F32, tag="l120")
    nc.vector.memset(l120, 0.0)
    for side in range(2):
        for i in range(3):
            # dst cols (b, d) at fixed (side, i); src r_dram[j, i*3+d]
            nc.gpsimd.dma_start(
                out=bass.AP(
                    tensor=l120.tensor,
                    offset=l120.offset + side * 60 + i * 5,
                    ap=[l120.ap[0], [15, 4], [1, 3]],
                ),
                in_=bass.AP(
                    tensor=r_dram.tensor,
                    offset=r_dram.offset + side * 36 + i * 3,
                    ap=[[0, C], [9, 4], [1, 3]],
                ),
            )
    oh_b = bass.AP(
        tensor=oh21.tensor, offset=oh21.offset,
        ap=[oh21.ap[0], [0, 2], [1, 4], [0, 15]],
    )
    nc.vector.tensor_tensor(out=l120, in0=l120[:], in1=oh_b, op=OP.mult)
    # fold the -2 of the d2 cross term into the pred-side transform
    nc.vector.tensor_scalar_mul(l120[:, 0:60], l120[:, 0:60], -2.0)

    # expand to the quadrant-padded K=96 layout via a DRAM bounce:
    # l96p[32*i + c, side*20 + b*5 + d] (pad rows/cols zero), fp16
    l_dram = pool["dram"].tile([C, 120], F32, tag="l_dram")
    nc.sync.dma_start(out=l_dram, in_=l120)
    l96p = setup.tile([96, 40], F16, tag="l96p")
    nc.vector.memset(l96p, 0.0)
    for i in range(3):
        for side in range(2):
            nc.gpsimd.dma_start(
                out=bass.AP(
                    tensor=l96p.tensor,
                    offset=l96p.offset + (32 * i) * 40 + side * 20,
                    ap=[[40, C], [5, 4], [1, 3]],
                ),
                in_=bass.AP(
                    tensor=l_dram.tensor,
                    offset=l_dram.offset + side * 60 + i * 5,
                    ap=[[120, C], [15, 4], [1, 3]],
                ),
            )

    # quadrant-padded fp16 vertex table: rows 32*i + c = coordinate plane i.
    # Strided DMA degenerates to per-element descriptors, so the plane
    # de-interleave runs on the PE instead: identity matmuls whose strided
    # *rhs* views pull out each coordinate plane, landing at psum bases 0/32/64.
    vnat = setup.tile([C, V * 3], F16, tag="vnat")
    nc.gpsimd.dma_start(out=vnat, in_=h["verts"].ap())
    vview = vnat[:].rearrange("c (v i) -> c v i", i=3)
    id21 = setup.tile([C, C], F16, tag="id21")
    nc.sync.dma_start(out=id21, in_=h["id21"].ap())
    table96 = setup.tile([96, V], F16, tag="table96")
    nc.gpsimd.memset(table96, 0.0)
    for n in range(4):
        nsl = slice(n * 512, (n + 1) * 512)
        ptb = psS.tile([96, 512], F32, tag="small")
        for i in range(3):
            nc.tensor.matmul(
                ptb[32 * i : 32 * i + C, :], lhsT=id21, rhs=vview[:, nsl, i : i + 1],
                start=True, stop=True, skip_group_check=True,
            )
            nc.scalar.copy(
                out=table96[32 * i : 32 * i + C, nsl],
                in_=ptb[32 * i : 32 * i + C, :],
            )

    # ---------------- sym flags ----------------
    sym_i = setup.tile([21, 1], I32, tag="sym_i")
    nc.sync.dma_start(out=sym_i, in_=h["sym"].ap())
    sym_f = setup.tile([21, 1], F32, tag="sym_f")
    nc.vector.tensor_copy(out=sym_f, in_=sym_i)
    ps_sym = psS.tile([1, 4], F32, tag="small")
    nc.tensor.matmul(ps_sym, lhsT=sym_f, rhs=oh21, start=True, stop=True)
    sym_row = acc.tile([1, 4], F32, tag="sym_row")
    nc.vector.tensor_copy(out=sym_row, in_=ps_sym)

    # ---------------- confidence loss: sum softplus(-x) ----------------
    e_scr = setup.tile([BPC, NCONF], F32, tag="e_scr")
    nc.scalar.activation(out=e_scr, in_=conf, func=AF.Exp, scale=-1.0)
    ln_scr = setup.tile([BPC, NCONF], F32, tag="ln_scr")
    sp_acc = setup.tile([BPC, 1], F32, tag="sp_acc")
    nc.scalar.activation(
        out=ln_scr, in_=e_scr, func=AF.Ln, bias=1.0, accum_out=sp_acc
    )
    ps_sp = psS.tile([1, 1], F32, tag="small")
    nc.tensor.matmul(ps_sp, lhsT=sp_acc, rhs=ones[0:BPC, :], start=True, stop=True)
    sp_sum = acc.tile([1, 1], F32, tag="sp_sum")
    nc.vector.tensor_copy(out=sp_sum, in_=ps_sp)

    # ---------------- pose regularization ----------------
    tsq = setup.tile([3, 4], F32, tag="tsq")
    nc.scalar.activation(out=tsq, in_=t5[0:3, 0:4], func=AF.Square)
    ps_tn = psS.tile([1, 4], F32, tag="small")
    nc.tensor.matmul(ps_tn, lhsT=ones[0:3, :], rhs=tsq, start=True, stop=True)
    tn = setup.tile([1, 4], F32, tag="tn")
    nc.scalar.activation(out=tn, in_=ps_tn, func=AF.Sqrt)
    bias_m2 = setup.tile([1, 1], F32, tag="bias_m2")
    nc.vector.memset(bias_m2, -2.0)
    rr = setup.tile([1, 4], F32, tag="rr")
    nc.scalar.activation(out=rr, in_=tn, func=AF.Relu, bias=bias_m2)
    rsq = setup.tile([1, 4], F32, tag="rsq")
    pr_acc = acc.tile([1, 1], F32, tag="pr_acc")
    nc.scalar.activation(out=rsq, in_=rr, func=AF.Square, accum_out=pr_acc)

    # ---------------- accumulators for the main loop ----------------
    colmin = acc.tile([128, BPC * 32], F32, tag="colmin")  # (b, m, half)
    addacc = acc.tile([1, BPC * 4], F32, tag="addacc")  # (b, nchunk)

    # pre-zero the small-psum slots so the ones-row scale=0 trick below never
    # multiplies uninitialized (possibly NaN) PSUM bits
    pz0 = psS.tile([128, 512], F32, tag="small")
    nc.vector.memset(pz0, 0.0)
    pz1 = psS.tile([128, 512], F32, tag="small")
    nc.vector.memset(pz1, 0.0)

    # ---------------- main per-batch loop ----------------
    for b in range(BPC):
        # a5 rows: [-2(p+t) x3, 1, pn];  g5 rows: [(g+t) x3, gn, 1]  (fp16)
        # rows 32-36 hold a DMA-replicated copy so alternate d2 m-chunks run
        # in PE row-group 32 (subarray concurrency + LDWEIGHTS pull-ahead)
        a5 = pool["ab"].tile([37, V], F16, tag="a5")
        g5 = pool["ab"].tile([37, V], F16, tag="g5")

        for side in (1, 0):  # gt first so the d2 matmuls can start earlier
            j = side * 4 + b
            dst = a5 if side == 0 else g5
            for n in range(4):
                nsl = slice(n * 512, (n + 1) * 512)
                p5 = psS.tile([5, 512], F32, tag="small")
                # K=96 transform+gather matmul; lhsT cols 3-4 are zero so
                # psum rows 3-4 get 0 (pred-side L carries the -2 factor)
                nc.tensor.matmul(
                    p5,
                    lhsT=l96p[:, side * 20 + b * 5 : side * 20 + (b + 1) * 5],
                    rhs=table96[:, nsl],
                    start=True,
                    stop=True,
                )
                # squared true point coords (for pn / gn)
                sqc = work.tile([3, 512], F16, tag="sqc")
                nc.scalar.activation(
                    out=sqc, in_=p5[0:3, :], func=AF.Square,
                    bias=t5[0:3, j : j + 1],
                    scale=-0.5 if side == 0 else 1.0,
                )
                # norm row fill: pred row 4 <- pn, gt row 3 <- gn.  The ones
                # row is synthesized by the copy-out below (scale 0, bias 1).
                nc.tensor.matmul(
                    p5, lhsT=(e3x5_r4 if side == 0 else e3x5_r3), rhs=sqc,
                    start=False, stop=True, skip_group_check=True,
                )
                nc.scalar.activation(
                    out=dst[0:5, nsl], in_=p5, func=AF.Identity,
                    bias=(bias_a if side == 0 else bias_g)[:, j : j + 1],
                    scale=(scale_a if side == 0 else scale_g)[:, 0:1],
                )

        # replicate rows 0-4 into rows 32-36 via a contiguous DRAM bounce
        for dst in (a5, g5):
            rep = pool["dram"].tile([5, V], F16, tag="rep")
            nc.sync.dma_start(out=rep, in_=dst[0:5, :])
            nc.sync.dma_start(out=dst[32:37, :], in_=rep)

        # ---- ADD (corresponding-point distance), true-difference form ----
        # u = 0.5*a5 + g5 = -(p+t_p) + (g+t_g) = -diff;  sum_d u^2 = |diff|^2
        h2 = work.tile([3, V], F32, tag="h2")
        nc.scalar.activation(out=h2, in_=a5[0:3, :], func=AF.Copy, scale=0.5)
        u = work.tile([3, V], F32, tag="u")
        nc.gpsimd.tensor_add(u, h2, g5[0:3, :])
        usq = work.tile([3, V], F16, tag="usq")
        nc.scalar.activation(out=usq, in_=u, func=AF.Square)
        for n in range(4):
            nsl = slice(n * 512, (n + 1) * 512)
            ps_da = psS.tile([1, 512], F32, tag="small")
            nc.tensor.matmul(
                ps_da, lhsT=ones3h, rhs=usq[:, nsl], start=True, stop=True
            )
            da_scr = work.tile([1, 512], F32, tag="da_scr")
            nc.scalar.activation(
                out=da_scr, in_=ps_da, func=AF.Sqrt,
                accum_out=addacc[:, b * 4 + n : b * 4 + n + 1],
            )

        # ---- ADD-S: pairwise (gn - 2 p.g) matmuls + column-min reduce ----
        for m in range(16):
            msl = slice(m * 128, (m + 1) * 128)
            for nh in range(2):
                d2 = psB.tile([128, 1024], F32, tag="d2")
                for ns in range(2):
                    off = nh * 1024 + ns * 512
                    nc.tensor.matmul(
                        d2[:, ns * 512 : (ns + 1) * 512],
                        lhsT=a5[32 * (m % 2) : 32 * (m % 2) + 5, msl],
                        rhs=g5[32 * (m % 2) : 32 * (m % 2) + 5, off : off + 512],
                        start=True,
                        stop=True,
                    )
                col = (b * 16 + m) * 2 + nh
                # colmin[:, col] = min_w(pn + gn - 2 p.g)
                nc.vector.tensor_reduce(
                    out=colmin[:, col : col + 1], in_=d2, axis=AX.X, op=OP.min
                )

    # ---------------- epilogue ----------------
    mins2 = work.tile([128, BPC * 16], F32, tag="mins2")
    nc.vector.tensor_reduce(
        out=mins2, in_=colmin[:].rearrange("p (c h) -> p c h", h=2),
        axis=AX.X, op=OP.min,
    )
    minsc = work.tile([128, BPC * 16], F32, tag="minsc")
    nc.vector.tensor_scalar_max(minsc, mins2, 1e-12)
    sqm = work.tile([128, BPC * 16], F32, tag="sqm")
    nc.scalar.activation(out=sqm, in_=minsc, func=AF.Sqrt)
    ps_adds = psS.tile([1, BPC * 16], F32, tag="small")
    nc.tensor.matmul(ps_adds, lhsT=ones, rhs=sqm, start=True, stop=True)
    adds_s = work.tile([1, BPC], F32, tag="adds_s")
    nc.vector.tensor_reduce(
        out=adds_s, in_=ps_adds[:].rearrange("p (b m) -> p b m", b=BPC),
        axis=AX.X, op=OP.add,
    )
    adds_a = work.tile([1, BPC], F32, tag="adds_a")
    nc.vector.tensor_reduce(
        out=adds_a, in_=addacc[:].rearrange("p (b n) -> p b n", b=BPC),
        axis=AX.X, op=OP.add,
    )
    # sel = adds_a + sym * (adds_s - adds_a)
    dlt = work.tile([1, BPC], F32, tag="dlt")
    nc.vector.tensor_sub(dlt, adds_s, adds_a)
    dls = work.tile([1, BPC], F32, tag="dls")
    nc.vector.tensor_mul(dls, dlt, sym_row)
    sel = work.tile([1, BPC], F32, tag="sel")
    nc.vector.tensor_add(sel, adds_a, dls)
    selsum = work.tile([1, 1], F32, tag="selsum")
    nc.vector.tensor_reduce(out=selsum, in_=sel, axis=AX.X, op=OP.add)

    out_sb = acc.tile([1, 4], F32, tag="out_sb")
    nc.vector.tensor_copy(out=out_sb[:, 0:1], in_=selsum)
    nc.vector.tensor_copy(out=out_sb[:, 1:2], in_=sp_sum)
    nc.vector.tensor_copy(out=out_sb[:, 2:3], in_=pr_acc)
    nc.vector.memset(out_sb[:, 3:4], 0.0)
    nc.sync.dma_start(out=h["out"].ap(), in_=out_sb[:])


def build_nc():
    nc = bacc.Bacc("TRN2", target_bir_lowering=False, debug=False)
    h = {}
    h["poses"] = nc.dram_tensor("poses", [8, 7], F32, kind="ExternalInput")
    h["conf"] = nc.dram_tensor("conf", [BPC, NCONF], F32, kind="ExternalInput")
    h["cls"] = nc.dram_tensor("cls", [BPC], I32, kind="ExternalInput")
    h["verts"] = nc.dram_tensor("verts", [C, V, 3], F32, kind="ExternalInput")
    h["sym"] = nc.dram_tensor("sym", [C], I32, kind="ExternalInput")
    h["out"] = nc.dram_tensor("partial", [1, 4], F32, kind="ExternalOutput")
    h["iota21"] = nc.inline_tensor(
        np.arange(C, dtype=np.float32).reshape(21, 1), "iota21"
    )
    h["ones"] = nc.inline_tensor(np.ones((128, 1), np.float32), "ones128")
    h["ones3h"] = nc.inline_tensor(np.ones((3, 1), np.float16), "ones3h")
    h["id21"] = nc.inline_tensor(np.eye(C, dtype=np.float16), "id21")
    h["mask_a"] = nc.inline_tensor(
        np.array([[-2.0], [-2.0], [-2.0], [0.0], [0.0]], np.float32), "mask_a"
    )
    h["mask_g"] = nc.inline_tensor(
        np.array([[1.0], [1.0], [1.0], [0.0], [0.0]], np.float32), "mask_g"
    )
    # a5 ones row = 3 (bias 1, scale 0); pn row = 4.  g5: gn row 3, ones row 4.
    h["scale_a"] = nc.inline_tensor(
        np.array([[1.0], [1.0], [1.0], [0.0], [1.0]], np.float32), "scale_a"
    )
    h["scale_g"] = nc.inline_tensor(
        np.array([[1.0], [1.0], [1.0], [1.0], [0.0]], np.float32), "scale_g"
    )
    h["addv_a"] = nc.inline_tensor(
        np.array([[0.0], [0.0], [0.0], [1.0], [0.0]], np.float32), "addv_a"
    )
    h["addv_g"] = nc.inline_tensor(
        np.array([[0.0], [0.0], [0.0], [0.0], [1.0]], np.float32), "addv_g"
    )
    for name, rows, col in (("e3x5_r3", 3, 3), ("e3x5_r4", 3, 4)):
        e = np.zeros((rows, 5), np.float16)
        e[:, col] = 1.0
        h[name] = nc.inline_tensor(e, name)

    with tile.TileContext(nc) as tc, ExitStack() as ctx:
        _emit(nc, tc, h, ctx)
    nc.compile()
    return nc


def make_in_maps(pred_poses, gt_poses, pred_confidences, model_vertices, class_ids, sym_mask):
    pred_poses = np.asarray(pred_poses, np.float32)
    gt_poses = np.asarray(gt_poses, np.float32)
    pred_confidences = np.asarray(pred_confidences, np.float32)
    model_vertices = np.ascontiguousarray(np.asarray(model_vertices, np.float32))
    class_ids = np.asarray(class_ids, np.int32)
    sym_mask = np.asarray(sym_mask, np.int32)
    in_maps = []
    for i in range(NCORES):
        s = slice(i * BPC, (i + 1) * BPC)
        in_maps.append(
            {
                "poses": np.ascontiguousarray(
                    np.concatenate([pred_poses[s], gt_poses[s]], axis=0)
                ),
                "conf": np.ascontiguousarray(pred_confidences[s]),
                "cls": np.ascontiguousarray(class_ids[s]),
                "verts": model_vertices,
                "sym": sym_mask,
            }
        )
    return in_maps


def combine_partials(partials):
    partials = np.asarray(partials, np.float64)
    add_total = partials[:, 0].sum() / (B * V)
    conf_total = partials[:, 1].sum() / (B * NCONF)
    reg_total = partials[:, 2].sum() / B
    total = ADD_WEIGHT * add_total + CONF_WEIGHT * conf_total + POSE_REG_WEIGHT * reg_total
    return np.array(total, dtype=np.float32)


def kernel(**inputs):
    if "nc" not in _CACHE:
        _CACHE["nc"] = build_nc()
    nc = _CACHE["nc"]
    in_maps = make_in_maps(**inputs)
    res = run_bass_kernel_spmd(nc, in_maps, list(range(NCORES)))
    partials = np.stack([res.results[i]["partial"][0] for i in range(NCORES)])
    return combine_partials(partials)


# revision 64
# speedup vs baseline: 1.0669x; 1.0669x over previous
"""Trainium2 Bass kernel for nn_DenseFusionLoss (DenseFusion pose-estimation loss).

Strategy: data-parallel over the batch axis. Each of the 8 NeuronCores gets 4
batches (poses/confidences/class_ids shard) plus the full replicated
[21,2048,3] vertex table. Each core computes partial sums
[sum_selected_add_loss, sum_softplus, sum_pose_reg]; the host combines the 8
partial vectors into the final scalar loss.

Device-side computation per core (all heavy math on-device):
  - quat -> rotation matrices via unnormalized-product form (scaled by 1/|q|^2)
  - vertex gather fused into the point-transform matmul: lhsT[(c,i), d] =
    onehot[c] * R[d,i] (K=63), rhs = vertex table laid out [63, 2048]
  - pairwise squared distances d2[v,w] = pn[v] + gn[w] - 2 p.g via a K=5
    matmul with lhsT rows [-2px,-2py,-2pz, pn, 1], rhs rows [gx,gy,gz, 1, gn]
  - ADD-S: DVE reduce_min over PSUM d2 tiles, clamp, sqrt, mean
  - ADD: true-difference form on GPSIMD + PE partition-sum + ACT sqrt-accum
  - conf loss: softplus(-x) = Ln(1 + Exp(-x)) with ACT accumulate
  - pose reg: relu(|t|-2)^2 via ACT
"""

from contextlib import ExitStack

import numpy as np

import concourse.bass as bass
import concourse.bacc as bacc
import concourse.tile as tile
from concourse import mybir
from concourse.bass_utils import run_bass_kernel_spmd

B, C, V, NCONF = 32, 21, 2048, 1024
NCORES = 8
BPC = B // NCORES  # batches per core
F32 = mybir.dt.float32
F16 = mybir.dt.float16
I32 = mybir.dt.int32
AF = mybir.ActivationFunctionType
OP = mybir.AluOpType
AX = mybir.AxisListType

ADD_WEIGHT = 1.0
CONF_WEIGHT = 0.1
POSE_REG_WEIGHT = 0.1

_CACHE = {}


def _emit(nc, tc, h, ctx):
    pool = {}
    pool["setup"] = ctx.enter_context(tc.tile_pool(name="setup", bufs=1))
    pool["acc"] = ctx.enter_context(tc.tile_pool(name="acc", bufs=1))
    pool["ab"] = ctx.enter_context(tc.tile_pool(name="ab", bufs=2))
    pool["work"] = ctx.enter_context(tc.tile_pool(name="work", bufs=2))
    pool["psB"] = ctx.enter_context(tc.tile_pool(name="psB", bufs=3, space="PSUM"))
    pool["psS"] = ctx.enter_context(tc.tile_pool(name="psS", bufs=2, space="PSUM"))
    pool["dram"] = ctx.enter_context(tc.tile_pool(name="dram", bufs=2, space="DRAM"))

    setup = pool["setup"]
    acc = pool["acc"]
    work = pool["work"]
    psS = pool["psS"]
    psB = pool["psB"]

    # ---------------- constant / input loads ----------------
    iota21 = setup.tile([21, 1], F32, tag="iota21")
    nc.sync.dma_start(out=iota21, in_=h["iota21"].ap())
    ones = setup.tile([128, 1], F32, tag="ones")
    nc.sync.dma_start(out=ones, in_=h["ones"].ap())



    poses = setup.tile([8, 7], F32, tag="poses")
    nc.sync.dma_start(out=poses, in_=h["poses"].ap())

    # t5[d, j] = poses[j, d] for d in 0..4 (rows 3-4 junk, masked to 0 below)
    t5 = setup.tile([5, 8], F32, tag="t5")
    for j in range(8):
        nc.sync.dma_start(
            out=t5[:, j : j + 1],
            in_=bass.AP(tensor=h["poses"].ap().tensor, offset=j * 7, ap=[[1, 5]]),
        )
    mask_a = setup.tile([5, 1], F32, tag="mask_a")
    nc.sync.dma_start(out=mask_a, in_=h["mask_a"].ap())
    mask_g = setup.tile([5, 1], F32, tag="mask_g")
    nc.sync.dma_start(out=mask_g, in_=h["mask_g"].ap())
    # fill matmul lhsT constants: e3x5_rK sums 3 rows into psum row K
    e3x5_r3 = setup.tile([3, 5], F16, tag="e3x5_r3")
    nc.sync.dma_start(out=e3x5_r3, in_=h["e3x5_r3"].ap())
    e3x5_r4 = setup.tile([3, 5], F16, tag="e3x5_r4")
    nc.sync.dma_start(out=e3x5_r4, in_=h["e3x5_r4"].ap())
    ones3h = setup.tile([3, 1], F16, tag="ones3h")
    nc.sync.dma_start(out=ones3h, in_=h["ones3h"].ap())
    # per-row copy-out scale (zero on the constant-ones row) and bias
    # columns: rows 0-2 = (-2|1)*t, ones row = 1, norm row = 0
    scale_a = setup.tile([5, 1], F32, tag="scale_a")
    nc.sync.dma_start(out=scale_a, in_=h["scale_a"].ap())
    scale_g = setup.tile([5, 1], F32, tag="scale_g")
    nc.sync.dma_start(out=scale_g, in_=h["scale_g"].ap())
    addv_a = setup.tile([5, 1], F32, tag="addv_a")
    nc.sync.dma_start(out=addv_a, in_=h["addv_a"].ap())
    addv_g = setup.tile([5, 1], F32, tag="addv_g")
    nc.sync.dma_start(out=addv_g, in_=h["addv_g"].ap())
    bias_a = setup.tile([5, 8], F32, tag="bias_a")
    nc.vector.tensor_scalar(
        out=bias_a, in0=t5, scalar1=mask_a, scalar2=addv_a,
        op0=OP.mult, op1=OP.add,
    )
    bias_g = setup.tile([5, 8], F32, tag="bias_g")
    nc.vector.tensor_scalar(
        out=bias_g, in0=t5, scalar1=mask_g, scalar2=addv_g,
        op0=OP.mult, op1=OP.add,
    )

    conf = setup.tile([BPC, NCONF], F32, tag="conf")
    nc.sync.dma_start(out=conf, in_=h["conf"].ap())

    # ---------------- quaternion -> rotation matrices ----------------
    q = poses[:, 3:7]
    qsq = setup.tile([8, 4], F32, tag="qsq")
    nc.vector.tensor_mul(qsq, q, q)
    nrm2 = setup.tile([8, 1], F32, tag="nrm2")
    nc.vector.tensor_reduce(out=nrm2, in_=qsq, axis=AX.X, op=OP.add)
    inv2 = setup.tile([8, 1], F32, tag="inv2")
    nc.vector.reciprocal(inv2, nrm2)
    s2 = setup.tile([8, 1], F32, tag="s2")
    nc.vector.tensor_scalar_mul(s2, inv2, 2.0)
    ns2 = setup.tile([8, 1], F32, tag="ns2")
    nc.vector.tensor_scalar_mul(ns2, inv2, -2.0)

    # cross products: xy xz yz wx wy wz
    pr = setup.tile([8, 6], F32, tag="pr")
    nc.vector.tensor_mul(pr[:, 0:1], q[:, 1:2], q[:, 2:3])  # xy
    nc.vector.tensor_mul(pr[:, 1:2], q[:, 1:2], q[:, 3:4])  # xz
    nc.vector.tensor_mul(pr[:, 2:3], q[:, 2:3], q[:, 3:4])  # yz
    nc.vector.tensor_mul(pr[:, 3:4], q[:, 0:1], q[:, 1:2])  # wx
    nc.vector.tensor_mul(pr[:, 4:5], q[:, 0:1], q[:, 2:3])  # wy
    nc.vector.tensor_mul(pr[:, 5:6], q[:, 0:1], q[:, 3:4])  # wz

    xx, yy, zz = qsq[:, 1:2], qsq[:, 2:3], qsq[:, 3:4]
    xy, xz, yz = pr[:, 0:1], pr[:, 1:2], pr[:, 2:3]
    wx, wy, wz = pr[:, 3:4], pr[:, 4:5], pr[:, 5:6]

    sm = setup.tile([8, 9], F32, tag="sm")
    # entry order is column-major: e = i*3 + d holds R[d][i], so that
    # (b, d)-indexed DMA reads of r_dram have unit stride in d.
    # entry: (a op b); diag entries get ns2*sum + 1, off-diag s2*sum
    entries = [
        (yy, zz, OP.add, True),   # e=0: R00 = 1 - 2(yy+zz)/n2
        (xy, wz, OP.add, False),  # e=1: R10
        (xz, wy, OP.subtract, False),  # e=2: R20
        (xy, wz, OP.subtract, False),  # e=3: R01
        (xx, zz, OP.add, True),   # e=4: R11
        (yz, wx, OP.add, False),  # e=5: R21
        (xz, wy, OP.add, False),  # e=6: R02
        (yz, wx, OP.subtract, False),  # e=7: R12
        (xx, yy, OP.add, True),   # e=8: R22
    ]
    r_all = setup.tile([8, 9], F32, tag="r_all")
    for e, (a, b_, op, diag) in enumerate(entries):
        nc.vector.tensor_tensor(out=sm[:, e : e + 1], in0=a, in1=b_, op=op)
        nc.vector.tensor_scalar(
            out=r_all[:, e : e + 1],
            in0=sm[:, e : e + 1],
            scalar1=ns2 if diag else s2,
            scalar2=1.0 if diag else 0.0,
            op0=OP.mult,
            op1=OP.add,
        )

    # ---------------- one-hot class rows ----------------
    cls21 = setup.tile([21, 4], I32, tag="cls21")
    nc.gpsimd.dma_start(
        out=cls21,
        in_=bass.AP(tensor=h["cls"].ap().tensor, offset=0, ap=[[0, 21], [1, 4]]),
    )
    cls21f = setup.tile([21, 4], F32, tag="cls21f")
    nc.vector.tensor_copy(out=cls21f, in_=cls21)
    oh21 = setup.tile([21, 4], F32, tag="oh21")
    nc.vector.tensor_scalar(
        out=oh21, in0=cls21f, scalar1=iota21, scalar2=None, op0=OP.is_equal
    )

    # ---------------- replicated transform lhsT ----------------
    # l120[c, col], col = side*60 + b*15 + i*5 + d (d-slots of width 5, the
    # last two zero) holds onehot_b(c) * R_side,b[d, i] (pred side * -2)
    r_dram = pool["dram"].tile([8, 9], F32, tag="r_dram")
    nc.sync.dma_start(out=r_dram, in_=r_all)
    l120 = setup.tile([C, 120], F32, tag="l120")
    nc.vector.memset(l120, 0.0)
    for side in range(2):
        for i in range(3):
            # dst cols (b, d) at fixed (side, i); src r_dram[j, i*3+d]
            nc.gpsimd.dma_start(
                out=bass.AP(
                    tensor=l120.tensor,
                    offset=l120.offset + side * 60 + i * 5,
                    ap=[l120.ap[0], [15, 4], [1, 3]],
                ),
                in_=bass.AP(
                    tensor=r_dram.tensor,
                    offset=r_dram.offset + side * 36 + i * 3,
                    ap=[[0, C], [9, 4], [1, 3]],
                ),
            )
    oh_b = bass.AP(
        tensor=oh21.tensor, offset=oh21.offset,
        ap=[oh21.ap[0], [0, 2], [1, 4], [0, 15]],
    )
    nc.vector.tensor_tensor(out=l120, in0=l120[:], in1=oh_b, op=OP.mult)
    # fold the -2 of the d2 cross term into the pred-side transform
    nc.vector.tensor_scalar_mul(l120[:, 0:60], l120[:, 0:60], -2.0)

    # expand to the quadrant-padded K=96 layout via a DRAM bounce:
    # l96p[32*i + c, side*20 + b*5 + d] (pad rows/cols zero), fp16
    l_dram = pool["dram"].tile([C, 120], F32, tag="l_dram")
    nc.sync.dma_start(out=l_dram, in_=l120)
    l96p = setup.tile([96, 40], F16, tag="l96p")
    nc.vector.memset(l96p, 0.0)
    for i in range(3):
        for side in range(2):
            nc.gpsimd.dma_start(
                out=bass.AP(
                    tensor=l96p.tensor,
                    offset=l96p.offset + (32 * i) * 40 + side * 20,
                    ap=[[40, C], [5, 4], [1, 3]],
                ),
                in_=bass.AP(
                    tensor=l_dram.tensor,
                    offset=l_dram.offset + side * 60 + i * 5,
                    ap=[[120, C], [15, 4], [1, 3]],
                ),
            )

    # quadrant-padded fp16 vertex table: rows 32*i + c = coordinate plane i.
    # Strided DMA degenerates to per-element descriptors, so the plane
    # de-interleave runs on the PE instead: identity matmuls whose strided
    # *rhs* views pull out each coordinate plane, landing at psum bases 0/32/64.
    vnat = setup.tile([C, V * 3], F16, tag="vnat")
    nc.gpsimd.dma_start(out=vnat, in_=h["verts"].ap())
    vview = vnat[:].rearrange("c (v i) -> c v i", i=3)
    id21 = setup.tile([C, C], F16, tag="id21")
    nc.sync.dma_start(out=id21, in_=h["id21"].ap())
    table96 = setup.tile([96, V], F16, tag="table96")
    nc.gpsimd.memset(table96, 0.0)
    for n in range(4):
        nsl = slice(n * 512, (n + 1) * 512)
        ptb = psS.tile([96, 512], F32, tag="small")
        for i in range(3):
            nc.tensor.matmul(
                ptb[32 * i : 32 * i + C, :], lhsT=id21, rhs=vview[:, nsl, i : i + 1],
                start=True, stop=True, skip_group_check=True,
            )
            nc.scalar.copy(
                out=table96[32 * i : 32 * i + C, nsl],
                in_=ptb[32 * i : 32 * i + C, :],
            )

    # ---------------- sym flags ----------------
    sym_i = setup.tile([21, 1], I32, tag="sym_i")
    nc.sync.dma_start(out=sym_i, in_=h["sym"].ap())
    sym_f = setup.tile([21, 1], F32, tag="sym_f")
    nc.vector.tensor_copy(out=sym_f, in_=sym_i)
    ps_sym = psS.tile([1, 4], F32, tag="small")
    nc.tensor.matmul(ps_sym, lhsT=sym_f, rhs=oh21, start=True, stop=True)
    sym_row = acc.tile([1, 4], F32, tag="sym_row")
    nc.vector.tensor_copy(out=sym_row, in_=ps_sym)

    # ---------------- accumulators for the main loop ----------------
    colmin = acc.tile([128, BPC * 32], F32, tag="colmin")  # (b, m, half)
    addacc = acc.tile([1, BPC * 4], F32, tag="addacc")  # (b, nchunk)

    # pre-zero the small-psum slots so the ones-row scale=0 trick below never
    # multiplies uninitialized (possibly NaN) PSUM bits
    pz0 = psS.tile([128, 512], F32, tag="small")
    nc.vector.memset(pz0, 0.0)
    pz1 = psS.tile([128, 512], F32, tag="small")
    nc.vector.memset(pz1, 0.0)

    # ---------------- main per-batch loop ----------------
    for b in range(BPC):
        # a5 rows: [-2(p+t) x3, 1, pn];  g5 rows: [(g+t) x3, gn, 1]  (fp16)
        a5 = pool["ab"].tile([5, V], F16, tag="a5")
        g5 = pool["ab"].tile([5, V], F16, tag="g5")

        for side in (1, 0):  # gt first so the d2 matmuls can start earlier
            j = side * 4 + b
            dst = a5 if side == 0 else g5
            for n in range(4):
                nsl = slice(n * 512, (n + 1) * 512)
                p5 = psS.tile([5, 512], F32, tag="small")
                # K=96 transform+gather matmul; lhsT cols 3-4 are zero so
                # psum rows 3-4 get 0 (pred-side L carries the -2 factor)
                nc.tensor.matmul(
                    p5,
                    lhsT=l96p[:, side * 20 + b * 5 : side * 20 + (b + 1) * 5],
                    rhs=table96[:, nsl],
                    start=True,
                    stop=True,
                )
                # squared true point coords (for pn / gn)
                sqc = work.tile([3, 512], F16, tag="sqc")
                nc.scalar.activation(
                    out=sqc, in_=p5[0:3, :], func=AF.Square,
                    bias=t5[0:3, j : j + 1],
                    scale=-0.5 if side == 0 else 1.0,
                )
                # norm row fill: pred row 4 <- pn, gt row 3 <- gn.  The ones
                # row is synthesized by the copy-out below (scale 0, bias 1).
                nc.tensor.matmul(
                    p5, lhsT=(e3x5_r4 if side == 0 else e3x5_r3), rhs=sqc,
                    start=False, stop=True, skip_group_check=True,
                )
                nc.scalar.activation(
                    out=dst[0:5, nsl], in_=p5, func=AF.Identity,
                    bias=(bias_a if side == 0 else bias_g)[:, j : j + 1],
                    scale=(scale_a if side == 0 else scale_g)[:, 0:1],
                )

        # ---- ADD (corresponding-point distance), true-difference form ----
        # u = 0.5*a5 + g5 = -(p+t_p) + (g+t_g) = -diff;  sum_d u^2 = |diff|^2
        # chunked so each da matmul only waits for its own quarter
        for n in range(4):
            nsl = slice(n * 512, (n + 1) * 512)
            h2 = work.tile([3, 512], F32, tag="h2")
            nc.scalar.activation(out=h2, in_=a5[0:3, nsl], func=AF.Copy, scale=0.5)
            u = work.tile([3, 512], F32, tag="u")
            nc.gpsimd.tensor_add(u, h2, g5[0:3, nsl])
            usq = work.tile([3, 512], F16, tag="usq")
            nc.scalar.activation(out=usq, in_=u, func=AF.Square)
            ps_da = psS.tile([1, 512], F32, tag="small")
            nc.tensor.matmul(
                ps_da, lhsT=ones3h, rhs=usq, start=True, stop=True
            )
            da_scr = work.tile([1, 512], F32, tag="da_scr")
            nc.scalar.activation(
                out=da_scr, in_=ps_da, func=AF.Sqrt,
                accum_out=addacc[:, b * 4 + n : b * 4 + n + 1],
            )

        # ---- ADD-S: pairwise (gn - 2 p.g) matmuls + column-min reduce ----
        for m in range(16):
            msl = slice(m * 128, (m + 1) * 128)
            for nh in range(2):
                d2 = psB.tile([128, 1024], F32, tag="d2")
                for ns in range(2):
                    off = nh * 1024 + ns * 512
                    nc.tensor.matmul(
                        d2[:, ns * 512 : (ns + 1) * 512],
                        lhsT=a5[:, msl],
                        rhs=g5[:, off : off + 512],
                        start=True,
                        stop=True,
                    )
                col = (b * 16 + m) * 2 + nh
                # colmin[:, col] = min_w(pn + gn - 2 p.g)
                nc.vector.tensor_reduce(
                    out=colmin[:, col : col + 1], in_=d2, axis=AX.X, op=OP.min
                )

    # ---------------- confidence loss: sum softplus(-x) ----------------
    e_scr = setup.tile([BPC, NCONF], F32, tag="e_scr")
    nc.scalar.activation(out=e_scr, in_=conf, func=AF.Exp, scale=-1.0)
    ln_scr = setup.tile([BPC, NCONF], F32, tag="ln_scr")
    sp_acc = setup.tile([BPC, 1], F32, tag="sp_acc")
    nc.scalar.activation(
        out=ln_scr, in_=e_scr, func=AF.Ln, bias=1.0, accum_out=sp_acc
    )
    ps_sp = psS.tile([1, 1], F32, tag="small")
    nc.tensor.matmul(ps_sp, lhsT=sp_acc, rhs=ones[0:BPC, :], start=True, stop=True)
    sp_sum = acc.tile([1, 1], F32, tag="sp_sum")
    nc.vector.tensor_copy(out=sp_sum, in_=ps_sp)

    # ---------------- pose regularization ----------------
    tsq = setup.tile([3, 4], F32, tag="tsq")
    nc.scalar.activation(out=tsq, in_=t5[0:3, 0:4], func=AF.Square)
    ps_tn = psS.tile([1, 4], F32, tag="small")
    nc.tensor.matmul(ps_tn, lhsT=ones[0:3, :], rhs=tsq, start=True, stop=True)
    tn = setup.tile([1, 4], F32, tag="tn")
    nc.scalar.activation(out=tn, in_=ps_tn, func=AF.Sqrt)
    bias_m2 = setup.tile([1, 1], F32, tag="bias_m2")
    nc.vector.memset(bias_m2, -2.0)
    rr = setup.tile([1, 4], F32, tag="rr")
    nc.scalar.activation(out=rr, in_=tn, func=AF.Relu, bias=bias_m2)
    rsq = setup.tile([1, 4], F32, tag="rsq")
    pr_acc = acc.tile([1, 1], F32, tag="pr_acc")
    nc.scalar.activation(out=rsq, in_=rr, func=AF.Square, accum_out=pr_acc)

    # ---------------- epilogue ----------------
    mins2 = work.tile([128, BPC * 16], F32, tag="mins2")
    nc.vector.tensor_reduce(
        out=mins2, in_=colmin[:].rearrange("p (c h) -> p c h", h=2),
        axis=AX.X, op=OP.min,
    )
    minsc = work.tile([128, BPC * 16], F32, tag="minsc")
    nc.vector.tensor_scalar_max(minsc, mins2, 1e-12)
    sqm = work.tile([128, BPC * 16], F32, tag="sqm")
    nc.scalar.activation(out=sqm, in_=minsc, func=AF.Sqrt)
    ps_adds = psS.tile([1, BPC * 16], F32, tag="small")
    nc.tensor.matmul(ps_adds, lhsT=ones, rhs=sqm, start=True, stop=True)
    adds_s = work.tile([1, BPC], F32, tag="adds_s")
    nc.vector.tensor_reduce(
        out=adds_s, in_=ps_adds[:].rearrange("p (b m) -> p b m", b=BPC),
        axis=AX.X, op=OP.add,
    )
    adds_a = work.tile([1, BPC], F32, tag="adds_a")
    nc.vector.tensor_reduce(
        out=adds_a, in_=addacc[:].rearrange("p (b n) -> p b n", b=BPC),
        axis=AX.X, op=OP.add,
    )
    # sel = adds_a + sym * (adds_s - adds_a)
    dlt = work.tile([1, BPC], F32, tag="dlt")
    nc.vector.tensor_sub(dlt, adds_s, adds_a)
    dls = work.tile([1, BPC], F32, tag="dls")
    nc.vector.tensor_mul(dls, dlt, sym_row)
    sel = work.tile([1, BPC], F32, tag="sel")
    nc.vector.tensor_add(sel, adds_a, dls)
    selsum = work.tile([1, 1], F32, tag="selsum")
    nc.vector.tensor_reduce(out=selsum, in_=sel, axis=AX.X, op=OP.add)

    out_sb = acc.tile([1, 4], F32, tag="out_sb")
    nc.vector.tensor_copy(out=out_sb[:, 0:1], in_=selsum)
    nc.vector.tensor_copy(out=out_sb[:, 1:2], in_=sp_sum)
    nc.vector.tensor_copy(out=out_sb[:, 2:3], in_=pr_acc)
    nc.vector.memset(out_sb[:, 3:4], 0.0)
    nc.sync.dma_start(out=h["out"].ap(), in_=out_sb[:])


def build_nc():
    nc = bacc.Bacc("TRN2", target_bir_lowering=False, debug=False)
    h = {}
    h["poses"] = nc.dram_tensor("poses", [8, 7], F32, kind="ExternalInput")
    h["conf"] = nc.dram_tensor("conf", [BPC, NCONF], F32, kind="ExternalInput")
    h["cls"] = nc.dram_tensor("cls", [BPC], I32, kind="ExternalInput")
    h["verts"] = nc.dram_tensor("verts", [C, V, 3], F32, kind="ExternalInput")
    h["sym"] = nc.dram_tensor("sym", [C], I32, kind="ExternalInput")
    h["out"] = nc.dram_tensor("partial", [1, 4], F32, kind="ExternalOutput")
    h["iota21"] = nc.inline_tensor(
        np.arange(C, dtype=np.float32).reshape(21, 1), "iota21"
    )
    h["ones"] = nc.inline_tensor(np.ones((128, 1), np.float32), "ones128")
    h["ones3h"] = nc.inline_tensor(np.ones((3, 1), np.float16), "ones3h")
    h["id21"] = nc.inline_tensor(np.eye(C, dtype=np.float16), "id21")
    h["mask_a"] = nc.inline_tensor(
        np.array([[-2.0], [-2.0], [-2.0], [0.0], [0.0]], np.float32), "mask_a"
    )
    h["mask_g"] = nc.inline_tensor(
        np.array([[1.0], [1.0], [1.0], [0.0], [0.0]], np.float32), "mask_g"
    )
    # a5 ones row = 3 (bias 1, scale 0); pn row = 4.  g5: gn row 3, ones row 4.
    h["scale_a"] = nc.inline_tensor(
        np.array([[1.0], [1.0], [1.0], [0.0], [1.0]], np.float32), "scale_a"
    )
    h["scale_g"] = nc.inline_tensor(
        np.array([[1.0], [1.0], [1.0], [1.0], [0.0]], np.float32), "scale_g"
    )
    h["addv_a"] = nc.inline_tensor(
        np.array([[0.0], [0.0], [0.0], [1.0], [0.0]], np.float32), "addv_a"
    )
    h["addv_g"] = nc.inline_tensor(
        np.array([[0.0], [0.0], [0.0], [0.0], [1.0]], np.float32), "addv_g"
    )
    for name, rows, col in (("e3x5_r3", 3, 3), ("e3x5_r4", 3, 4)):
        e = np.zeros((rows, 5), np.float16)
        e[:, col] = 1.0
        h[name] = nc.inline_tensor(e, name)

    with tile.TileContext(nc) as tc, ExitStack() as ctx:
        _emit(nc, tc, h, ctx)
    nc.compile()
    return nc


def make_in_maps(pred_poses, gt_poses, pred_confidences, model_vertices, class_ids, sym_mask):
    pred_poses = np.asarray(pred_poses, np.float32)
    gt_poses = np.asarray(gt_poses, np.float32)
    pred_confidences = np.asarray(pred_confidences, np.float32)
    model_vertices = np.ascontiguousarray(np.asarray(model_vertices, np.float32))
    class_ids = np.asarray(class_ids, np.int32)
    sym_mask = np.asarray(sym_mask, np.int32)
    in_maps = []
    for i in range(NCORES):
        s = slice(i * BPC, (i + 1) * BPC)
        in_maps.append(
            {
                "poses": np.ascontiguousarray(
                    np.concatenate([pred_poses[s], gt_poses[s]], axis=0)
                ),
                "conf": np.ascontiguousarray(pred_confidences[s]),
                "cls": np.ascontiguousarray(class_ids[s]),
                "verts": model_vertices,
                "sym": sym_mask,
            }
        )
    return in_maps


def combine_partials(partials):
    partials = np.asarray(partials, np.float64)
    add_total = partials[:, 0].sum() / (B * V)
    conf_total = partials[:, 1].sum() / (B * NCONF)
    reg_total = partials[:, 2].sum() / B
    total = ADD_WEIGHT * add_total + CONF_WEIGHT * conf_total + POSE_REG_WEIGHT * reg_total
    return np.array(total, dtype=np.float32)


def kernel(**inputs):
    if "nc" not in _CACHE:
        _CACHE["nc"] = build_nc()
    nc = _CACHE["nc"]
    in_maps = make_in_maps(**inputs)
    res = run_bass_kernel_spmd(nc, in_maps, list(range(NCORES)))
    partials = np.stack([res.results[i]["partial"][0] for i in range(NCORES)])
    return combine_partials(partials)
